# revision 37
# baseline (speedup 1.0000x reference)
"""AdaConv2D Trainium2 Bass kernel.

Problem (per sample): instance-norm(x) -> grouped 3x3 conv (128 groups,
2ch/group, per-sample weights) -> grouped 1x1 conv -> +bias.
B=8, Cin=Cout=256, H=W=128.

Strategy: pure data-parallel, 1 sample per NeuronCore (8 cores).

Per-core algorithm:
  - The 1x1 grouped conv is folded into the 3x3 weights:
        w_eff[co, j, t] = sum_i pw[co, i] * dw[2*(co//2)+i, j, t]
  - The instance norm is folded into weights + bias:
        lhsT[ci, co] = w_eff[co, j(ci), t] * scale[ci]
        bias'[co]    = bias[co] - sum_ci,t lhsT[ci, t, co] * mean[ci]
    where scale_c = 1/(sqrt(var_c)+eps); the padded border cells hold
    mean_c so that (border - mean)*scale = 0 matches the reference's
    zero-padded normalized input.
  - The grouped 3x3 conv runs on the TensorEngine as 9 shifted
    block-diagonal (2x2 blocks) 128x128 bf16 matmuls accumulated in PSUM,
    one pass per tap, channels on partitions (two halves of 128 channels).
  - Block-diag matrices: scatter the *unscaled* w_eff into a
    zero-initialized DRAM scratch (inline const) with strided DMAs (no
    stats dependency -> overlaps the x DMA-in), load dense [128,128]
    tiles back, then scale+cast per-partition (scale is indexed by ci =
    partition).  bias' comes from 9 accumulated N=1 matmuls of the scaled
    lhsT against mean[ci].
  - Per-half pipelining: half 0's conv overlaps half 1's input DMA.
"""

import sys

sys.path.insert(0, "/opt/trn_rl_repo")

from contextlib import ExitStack

import numpy as np
import ml_dtypes

from concourse import bacc, bass, mybir, tile
from concourse.bass_utils import run_bass_kernel_spmd

F32 = mybir.dt.float32
BF16 = mybir.dt.bfloat16
AX = mybir.AxisListType
OP = mybir.AluOpType
ACTF = mybir.ActivationFunctionType

C = 256          # channels (per sample)
H = W = 128      # spatial
P = 128          # partitions
HP = H + 2       # padded rows/cols (130)
NHF = 2          # channel halves
CHUNK_ROWS = 16  # rows per input DMA chunk
NCHUNK = H // CHUNK_ROWS          # 4 chunks per half
ROWS_PER_MM = 4                   # output rows per psum tile (4*128=512)
SB_TILES = 4                      # psum tiles per superblock
SB_ROWS = ROWS_PER_MM * SB_TILES  # 16 rows per superblock
NSB = H // SB_ROWS                # 8 superblocks per half
NPIX = H * W
EPS = 1e-7

_CACHED = {}


def build_nc():
    nc = bacc.Bacc(trn_type="TRN2")

    x_ext = nc.declare_dram_parameter("x", [C, H, W], F32, isOutput=False)
    dw_ext = nc.declare_dram_parameter("dw_kernels", [C, 2, 3, 3], F32, isOutput=False)
    pw_ext = nc.declare_dram_parameter("pw_kernels", [C, 2, 1, 1], F32, isOutput=False)
    b_ext = nc.declare_dram_parameter("biases", [C], F32, isOutput=False)
    out_ext = nc.declare_dram_parameter("out", [C, H, W], F32, isOutput=True)

    # zero-initialized DRAM scratch for the block-diag weight matrices;
    # runtime scatter only writes the (fixed) nonzero positions, so reuse
    # across executions is idempotent.  layout: [ci, hf, tap, co] f32
    # (ci-major so the load back to SBUF is one big descriptor per
    # partition instead of thousands of 512B ones)
    lhsT_dram = nc.inline_tensor(
        np.zeros((P, NHF, 9, P), dtype=np.float32), name="lhsT_zero"
    )

    with tile.TileContext(nc) as tc, ExitStack() as ctx:
        const_pool = ctx.enter_context(tc.tile_pool(name="const", bufs=1))
        chunk_pool = ctx.enter_context(tc.tile_pool(name="chunk", bufs=8))
        sq_pool = ctx.enter_context(tc.tile_pool(name="sq", bufs=2))
        psum_pool = ctx.enter_context(tc.tile_pool(name="psum", bufs=8, space="PSUM"))
        stage_pool = ctx.enter_context(tc.tile_pool(name="stage", bufs=4))

        # ---------------- persistent tiles ----------------
        xnp = [
            const_pool.tile([P, HP, HP], BF16, name=f"xnp{hf}") for hf in range(NHF)
        ]
        sums = const_pool.tile([P, NHF, NCHUNK], F32, name="sums")
        sumsqs = const_pool.tile([P, NHF, NCHUNK], F32, name="sumsqs")

        mean_ch = const_pool.tile([P, NHF], F32, name="mean_ch")
        mean_bf = const_pool.tile([P, NHF], BF16, name="mean_bf")
        scale_ch = const_pool.tile([P, NHF], F32, name="scale_ch")
        bias_ch = const_pool.tile([P, NHF], F32, name="bias_ch")
        biasp_ch = const_pool.tile([P, NHF], F32, name="biasp_ch")
        st_a = const_pool.tile([P, NHF], F32, name="st_a")
        st_b = const_pool.tile([P, NHF], F32, name="st_b")

        # group-layout weights (partition = group)
        dwg = const_pool.tile([P, 2, 2, 9], F32, name="dwg")    # [g, i, j, t]
        pwg = const_pool.tile([P, 2, 2], F32, name="pwg")       # [g, o, i]
        weff = const_pool.tile([P, 2, 2, 9], F32, name="weff")  # [g, o, j, t]

        # dense block-diag weights: raw f32 (unscaled) and scaled bf16
        lhsT_raw = const_pool.tile([P, NHF, 9, P], F32, name="lhsT_raw")
        lhsT_sb = const_pool.tile([P, NHF, 9, P], BF16, name="lhsT_sb")

        # ------------- early DMAs (no stats dependency) -------------
        nc.sync.dma_start(
            out=dwg[:],
            in_=bass.AP(tensor=dw_ext, offset=0, ap=[[36, P], [18, 2], [9, 2], [1, 9]]),
        )
        nc.sync.dma_start(
            out=pwg[:],
            in_=bass.AP(tensor=pw_ext, offset=0, ap=[[4, P], [2, 2], [1, 2]]),
        )

        # ------------- w_eff (group layout) + scatter + load -------------
        # at high priority so the scatter->load chain completes early in
        # the x DMA-in window (the DVE stream would otherwise order these
        # after all the chunk conversions)
        with tc.high_priority():
            for o in range(2):
                nc.vector.tensor_scalar(
                    out=weff[:, o],
                    in0=dwg[:, 0],
                    scalar1=pwg[:, o, 0:1],
                    scalar2=None,
                    op0=OP.mult,
                )
                nc.vector.scalar_tensor_tensor(
                    out=weff[:, o],
                    in0=dwg[:, 1],
                    scalar=pwg[:, o, 1:2],
                    in1=weff[:, o],
                    op0=OP.mult,
                    op1=OP.add,
                )
        # scatter: dst (ci=2a+j, hf, t, co=2a+o) <- weff[64*hf + a, o, j, t]
        # (DMA APs max out at 3 dims incl. the trailing unit -> one DMA
        #  per (hf, t, j) with dims (a, o))
        CI_STRIDE = NHF * 9 * P  # 2304

        def emit_scatter_load(hf, scatter_eng):
            for t in range(9):
                for j in range(2):
                    scatter_eng.dma_start(
                        out=bass.AP(
                            tensor=lhsT_dram,
                            offset=j * CI_STRIDE + hf * 9 * P + t * P,
                            ap=[[2 * CI_STRIDE + 2, 64], [1, 2]],
                        ),
                        in_=weff[64 * hf : 64 * (hf + 1), :, j, t],
                    )
            # load back densely: lhsT_raw[ci, hf, t, co] (contiguous 4.6KB
            # per partition)
            return nc.sync.dma_start(
                out=lhsT_raw[:, hf],
                in_=bass.AP(
                    tensor=lhsT_dram,
                    offset=hf * 9 * P,
                    ap=[[CI_STRIDE, P], [P, 9], [1, P]],
                ),
            )

        # x input chunks.  half 0 (latency-critical): split across both
        # HWDGE rings (SP + ACT) — ACT's stream is free pre-conv.  half 1:
        # SP ring only, so the ACT engine never blocks on DMA waits
        # mid-conv (that stalls epilogues -> PSUM banks -> TensorEngine).
        chunk_tiles = {0: [], 1: []}

        def emit_chunk(hf, ck):
            chv = chunk_pool.tile([P, CHUNK_ROWS, W], F32, name="chv")
            chunk_tiles[hf].append(chv)
            dma_eng = nc.scalar if (hf == 0 and ck % 2 == 1) else nc.sync
            return dma_eng.dma_start(
                out=chv[:],
                in_=x_ext[
                    hf * P : (hf + 1) * P,
                    ck * CHUNK_ROWS : (ck + 1) * CHUNK_ROWS,
                    :,
                ],
            )

        # ring/issue order: all h0 chunk issues first (so no chunk sits
        # behind the 18 scatter issues on the sync sequencer), then h0's
        # scatters+load.  h1's scatters+load run during conv h0.
        with tc.high_priority():
            for ck in range(NCHUNK):
                emit_chunk(0, ck)
            # h0 scatters issue from the ACT sequencer (free pre-conv) so
            # the sync ring's load DMA isn't stuck behind 18 issue slots
            load0_inst = emit_scatter_load(0, nc.scalar)
            # bias [256] -> bias_ch[c, hf]
            nc.sync.dma_start(
                out=bias_ch[:],
                in_=bass.AP(tensor=b_ext, offset=0, ap=[[1, P], [P, NHF]]),
            )

        # ------------- per-half pipeline -------------
        for hf in range(NHF):
            if hf == 1:
                for ck in range(NCHUNK):
                    inst = emit_chunk(1, ck)
                    if ck == 0:
                        # keep h1's 8 MiB off the DMA slots until h0's
                        # latency-critical weight load has completed
                        bass._add_dep_helper(
                            inst.ins,
                            load0_inst.ins,
                            sync=True,
                            reason="h1 x stream waits for h0 lhsT load",
                        )
                emit_scatter_load(1, nc.sync)
            for ck in range(NCHUNK):
                chv = chunk_tiles[hf][ck]
                # convert f32 -> bf16 into padded interior; accumulate sum
                nc.vector.tensor_scalar(
                    out=xnp[hf][
                        :, 1 + ck * CHUNK_ROWS : 1 + (ck + 1) * CHUNK_ROWS, 1 : 1 + W
                    ],
                    in0=chv[:],
                    scalar1=1.0,
                    scalar2=None,
                    op0=OP.mult,
                    op1=OP.add,
                    accum_out=sums[:, hf, ck : ck + 1],
                )
                # sum of squares via ScalarE
                sq = sq_pool.tile([P, CHUNK_ROWS, W], F32, name="sq")
                nc.scalar.activation(
                    out=sq[:],
                    in_=chv[:],
                    func=ACTF.Square,
                    accum_out=sumsqs[:, hf, ck : ck + 1],
                )

            # --- stats finalize (channel layout) ---
            nc.vector.tensor_reduce(
                out=st_a[:, hf : hf + 1], in_=sums[:, hf, :], axis=AX.X, op=OP.add
            )
            nc.vector.tensor_scalar(
                out=mean_ch[:, hf : hf + 1],
                in0=st_a[:, hf : hf + 1],
                scalar1=1.0 / NPIX,
                scalar2=None,
                op0=OP.mult,
            )
            nc.vector.tensor_reduce(
                out=st_a[:, hf : hf + 1], in_=sumsqs[:, hf, :], axis=AX.X, op=OP.add
            )
            nc.vector.tensor_tensor(
                out=st_b[:, hf : hf + 1],
                in0=mean_ch[:, hf : hf + 1],
                in1=mean_ch[:, hf : hf + 1],
                op=OP.mult,
            )
            nc.vector.scalar_tensor_tensor(
                out=st_b[:, hf : hf + 1],
                in0=st_b[:, hf : hf + 1],
                scalar=float(-NPIX),
                in1=st_a[:, hf : hf + 1],
                op0=OP.mult,
                op1=OP.add,
            )
            nc.vector.tensor_scalar(
                out=st_b[:, hf : hf + 1],
                in0=st_b[:, hf : hf + 1],
                scalar1=1.0 / (NPIX - 1),
                scalar2=None,
                op0=OP.mult,
            )
            nc.scalar.sqrt(st_b[:, hf : hf + 1], st_b[:, hf : hf + 1])
            nc.vector.tensor_scalar(
                out=st_b[:, hf : hf + 1],
                in0=st_b[:, hf : hf + 1],
                scalar1=EPS,
                scalar2=None,
                op0=OP.add,
            )
            nc.vector.reciprocal(scale_ch[:, hf : hf + 1], st_b[:, hf : hf + 1])
            nc.vector.tensor_copy(mean_bf[:, hf : hf + 1], mean_ch[:, hf : hf + 1])

            # --- scale + cast the block-diag weights (per-partition ci) ---
            nc.vector.tensor_scalar(
                out=lhsT_sb[:, hf],
                in0=lhsT_raw[:, hf],
                scalar1=scale_ch[:, hf : hf + 1],
                scalar2=None,
                op0=OP.mult,
            )

            # --- bias' = bias - lhsT^T @ mean  (9 accumulated N=1 matmuls) ---
            bps = psum_pool.tile([P, 1], F32, name="bps", tag="ps", bufs=8)
            for t in range(9):
                nc.tensor.matmul(
                    bps[:],
                    lhsT=lhsT_sb[:, hf, t, :],
                    rhs=mean_bf[:, hf : hf + 1],
                    start=(t == 0),
                    stop=(t == 8),
                )
            nc.vector.tensor_tensor(
                out=biasp_ch[:, hf : hf + 1],
                in0=bias_ch[:, hf : hf + 1],
                in1=bps[:],
                op=OP.subtract,
            )

            # --- border fill with mean (bf16) ---
            bias_ap = mean_ch[:, hf : hf + 1]
            nc.scalar.activation(
                out=xnp[hf][:, 1 : 1 + H, 0],
                in_=xnp[hf][:, 1 : 1 + H, 1],
                func=ACTF.Identity,
                bias=bias_ap,
                scale=0.0,
            )
            nc.scalar.activation(
                out=xnp[hf][:, 1 : 1 + H, HP - 1],
                in_=xnp[hf][:, 1 : 1 + H, 1],
                func=ACTF.Identity,
                bias=bias_ap,
                scale=0.0,
            )
            nc.scalar.activation(
                out=xnp[hf][:, 0, :],
                in_=xnp[hf][:, 1, :],
                func=ACTF.Identity,
                bias=bias_ap,
                scale=0.0,
            )
            nc.scalar.activation(
                out=xnp[hf][:, HP - 1, :],
                in_=xnp[hf][:, 1, :],
                func=ACTF.Identity,
                bias=bias_ap,
                scale=0.0,
            )

            # --- conv: 9 shifted block-diag matmuls per psum tile ---
            for sb in range(NSB):
                ps = [
                    psum_pool.tile([P, ROWS_PER_MM, W], F32, name="ps", tag="ps", bufs=8)
                    for _ in range(SB_TILES)
                ]
                for t in range(9):
                    dy, dx = t // 3, t % 3
                    for k in range(SB_TILES):
                        h0 = sb * SB_ROWS + k * ROWS_PER_MM
                        nc.tensor.matmul(
                            ps[k][:],
                            lhsT=lhsT_sb[:, hf, t, :],
                            rhs=xnp[hf][
                                :, h0 + dy : h0 + dy + ROWS_PER_MM, dx : dx + W
                            ],
                            start=(t == 0),
                            stop=(t == 8),
                        )
                # epilogue + store in 8-row blocks (2 psum tiles each) to
                # keep the kernel tail short
                for half_blk in range(2):
                    stg = stage_pool.tile([P, SB_ROWS // 2, W], F32, name="stg")
                    for kk in range(2):
                        k = half_blk * 2 + kk
                        nc.scalar.activation(
                            out=stg[:, kk * ROWS_PER_MM : (kk + 1) * ROWS_PER_MM, :],
                            in_=ps[k][:],
                            func=ACTF.Identity,
                            bias=biasp_ch[:, hf : hf + 1],
                            scale=1.0,
                        )
                    nc.sync.dma_start(
                        out=out_ext[
                            hf * P : (hf + 1) * P,
                            sb * SB_ROWS
                            + half_blk * (SB_ROWS // 2) : sb * SB_ROWS
                            + (half_blk + 1) * (SB_ROWS // 2),
                            :,
                        ],
                        in_=stg[:],
                    )

    nc.compile()
    return nc


def get_nc():
    if "nc" not in _CACHED:
        _CACHED["nc"] = build_nc()
    return _CACHED["nc"]


def kernel(x, dw_kernels, pw_kernels, biases):
    x = np.asarray(x, dtype=np.float32)
    dw_kernels = np.asarray(dw_kernels, dtype=np.float32)
    pw_kernels = np.asarray(pw_kernels, dtype=np.float32)
    biases = np.asarray(biases, dtype=np.float32)
    B = x.shape[0]
    assert B == 8

    nc = get_nc()
    in_maps = [
        {
            "x": np.ascontiguousarray(x[i]),
            "dw_kernels": np.ascontiguousarray(dw_kernels[i]),
            "pw_kernels": np.ascontiguousarray(pw_kernels[i]),
            "biases": np.ascontiguousarray(biases[i]),
        }
        for i in range(B)
    ]
    res = run_bass_kernel_spmd(nc, in_maps, core_ids=list(range(B)))
    return np.stack([res.results[i]["out"] for i in range(B)], axis=0)
